# revision 25
# baseline (speedup 1.0000x reference)
"""Trainium2 Bass kernel: 16-head causal attention (B=4, S=2048, E=1024).

Sharding: 8 cores = 4 batches x 2 head-groups (8 heads each).
Per-core, fp32 storage with float32r (FP22) matmuls at full PE rate:
  - q^T = Wq_g X^T, k^T = Wk_g X^T   (transposed projections, [dq, S])
  - V = X^T.T Wv_g^T                 (natural layout [S, dv], + ones column
    per head so the PV matmul also produces softmax denominators)
  - scores^T[k, q] blocks = k^T.T q^T (K=64 contraction, fp32r, N=512)
  - P^T = exp(scores^T/8 + causal additive mask)  (no max subtraction:
    scores are O(10), fp32 exp is safe; masked lanes underflow to 0)
  - out^T[d+1, q] += V_aug^T P^T accumulated in PSUM across k blocks;
    row d is the denominator. Normalize with a broadcast reciprocal.
  - partial = attnT.T @ Wo_g^T; host sums the two head-group partials + bo.
Causal: fully-masked k blocks are skipped; only the diagonal band gets
additive mask tiles (4 host-precomputed patterns).
"""

import contextlib

import numpy as np

import bass_rust
import concourse.bass as bass
import concourse.mybir as mybir
import concourse.tile as tile
from concourse.bass_utils import run_bass_kernel_spmd

F32 = mybir.dt.float32
F32R = mybir.dt.float32r
AF = mybir.ActivationFunctionType

B, S, E = 4, 2048, 1024
H, D = 16, 64
NCORES = 8
NGROUPS = 2            # head groups (tensor parallel)
HPC = H // NGROUPS     # heads per core
DQ = HPC * D           # per-core projection width = 512
NEG = -8.0e9           # additive mask; *0.125 in exp -> -1e9 -> exp == 0.0

SQ = 512               # q tile (free dim of scores^T)
SK = 128               # k block (partition dim of scores^T)
G = 2                  # k blocks per exp group (psum [128, G*512])


def r(ap):
    """View an fp32 AP as float32r for full-rate PE matmuls."""
    return ap.bitcast(F32R)


def split_excess_waits(nc, maxw=1):
    """This container's walrus supports one sem wait per instruction;
    hoist extras onto same-engine nops just before the instruction."""
    n_new = 0
    for bb in nc.main_func.blocks:
        new_list = []
        changed = False
        for inst in list(bb.instructions):
            si = inst.sync_info
            waits = list(si.on_wait) if si and si.on_wait else []
            if len(waits) > maxw:
                changed = True
                extra, keep = waits[:-maxw], waits[-maxw:]
                for ci in range(0, len(extra), maxw):
                    nop = bass_rust.InstNoOp(
                        name=f"I-waitsplit-{n_new}", ins=[], outs=[]
                    )
                    n_new += 1
                    nop.engine = inst.engine
                    nop.sync_info = mybir.SyncInfo(
                        on_wait=extra[ci : ci + maxw], on_update=[]
                    )
                    new_list.append(nop)
                inst.sync_info = mybir.SyncInfo(
                    on_wait=keep,
                    on_update=list(si.on_update) if si.on_update else [],
                )
            new_list.append(inst)
        if changed:
            bb.instructions = new_list
    return n_new


def build_kernel(s=S, e=E, hpc=HPC, d=D, causal=True, sq=SQ, g=G,
                 split_waits=True, debug=False):
    dq = hpc * d
    nec = e // 128            # input-feature chunks
    ndq = dq // 128           # projection partition chunks
    nsb = s // sq             # q blocks
    nsk = s // SK             # k blocks
    nsc = s // 128            # s chunks of 128
    ndiag = sq // SK          # diagonal mask patterns
    qw = min(sq, 512)         # matmul moving width
    nqw = sq // qw
    nxw = s // 512            # x piece columns

    nc = bass.Bass()

    xq = nc.declare_dram_parameter("xq_t", [e, s], F32R, isOutput=False)
    xk = nc.declare_dram_parameter("xk_t", [e, s], F32R, isOutput=False)
    xv = nc.declare_dram_parameter("xv_t", [e, s], F32R, isOutput=False)
    wqd = nc.declare_dram_parameter("wq_t", [e, dq], F32R, isOutput=False)
    wkd = nc.declare_dram_parameter("wk_t", [e, dq], F32R, isOutput=False)
    wvd = nc.declare_dram_parameter("wv_t", [e, dq], F32R, isOutput=False)
    wod = nc.declare_dram_parameter("wo_t", [dq, e], F32R, isOutput=False)
    bqd = nc.declare_dram_parameter("bq", [128, ndq], F32, isOutput=False)
    bkd = nc.declare_dram_parameter("bk", [128, ndq], F32, isOutput=False)
    bvd = nc.declare_dram_parameter("bv_b", [128, dq], F32, isOutput=False)
    if causal:
        mkd = nc.declare_dram_parameter(
            "maskadd", [128, sq + (ndiag - 1) * SK], F32, isOutput=False
        )
    else:
        mkd = nc.declare_dram_parameter(
            "maskadd_full", [s, s], F32, isOutput=False
        )
    onc = nc.declare_dram_parameter("ones_c", [1, d], F32R, isOutput=False)
    onv = nc.declare_dram_parameter("ones_v", [128, hpc], F32R, isOutput=False)
    out = nc.declare_dram_parameter("out", [s, e], F32, isOutput=True)
    if debug:
        dbg_q = nc.declare_dram_parameter("dbg_q", [dq, s], F32, isOutput=True)
        dbg_k = nc.declare_dram_parameter("dbg_k", [dq, s], F32, isOutput=True)
        dbg_v = nc.declare_dram_parameter(
            "dbg_v", [s, hpc * (d + 1)], F32, isOutput=True
        )
        dbg_at = nc.declare_dram_parameter(
            "dbg_at", [dq, s], F32, isOutput=True
        )

    with tile.TileContext(nc) as tc, contextlib.ExitStack() as ctx:
        pers = ctx.enter_context(tc.tile_pool(name="pers", bufs=1))
        wpool = ctx.enter_context(tc.tile_pool(name="wp", bufs=nec + 1))
        xpool = ctx.enter_context(tc.tile_pool(name="xp", bufs=nec + 1))
        wop = ctx.enter_context(tc.tile_pool(name="wop", bufs=ndq + 1))
        ppool = ctx.enter_context(tc.tile_pool(name="ppl", bufs=2))
        nrm = ctx.enter_context(tc.tile_pool(name="nrm", bufs=2))
        opool = ctx.enter_context(tc.tile_pool(name="opl", bufs=2))
        pp = ctx.enter_context(tc.tile_pool(name="pp", bufs=2, space="PSUM"))
        sp = ctx.enter_context(tc.tile_pool(name="sp", bufs=2, space="PSUM"))
        vp = ctx.enter_context(tc.tile_pool(name="vp", bufs=2, space="PSUM"))

        # ---- constants / persistent tensors ----
        bq_sb = pers.tile([128, ndq], F32, name="bq_sb")
        nc.sync.dma_start(out=bq_sb[:, :], in_=bqd[:, :])
        bk_sb = pers.tile([128, ndq], F32, name="bk_sb")
        nc.sync.dma_start(out=bk_sb[:, :], in_=bkd[:, :])
        bv_sb = pers.tile([128, dq], F32, name="bv_sb")
        nc.sync.dma_start(out=bv_sb[:, :], in_=bvd[:, :])
        ones_sb = pers.tile([1, d], F32R, name="ones_sb")
        nc.sync.dma_start(out=ones_sb[:, :], in_=onc[:, :])
        onv_sb = pers.tile([128, hpc], F32R, name="onv_sb")
        nc.sync.dma_start(out=onv_sb[:, :], in_=onv[:, :])
        if causal:
            # one shifted strip: pattern j at cols (ndiag-1-j)*SK
            mk_sb = pers.tile([128, sq + (ndiag - 1) * SK], F32, name="mk_sb")
            nc.sync.dma_start(out=mk_sb[:, :], in_=mkd[:, :])

        q_sb = [pers.tile([128, s], F32R, name=f"q_sb{c}") for c in range(ndq)]
        k_sb = [pers.tile([128, s], F32R, name=f"k_sb{c}") for c in range(ndq)]
        v_sb = [
            pers.tile([128, hpc * (d + 1)], F32R, name=f"v_sb{i}")
            for i in range(nsc)
        ]
        at_sb = [
            pers.tile([128, s], F32R, name=f"at_sb{c}") for c in range(ndq)
        ]

        # ---- phases 1+2: q^T / k^T projections (out [dq, s]) ----
        for which, wt, xt, dst, bias in (
            ("q", wqd, xq, q_sb, bq_sb),
            ("k", wkd, xk, k_sb, bk_sb),
        ):
            w_t = []
            for ec in range(nec):
                t = wpool.tile([128, dq], F32R, tag="w", name=f"w{which}{ec}")
                nc.sync.dma_start(
                    out=t[:, :], in_=wt[ec * 128 : (ec + 1) * 128, :]
                )
                w_t.append(t)
            for sb in range(nxw):
                x_t = []
                for ec in range(nec):
                    t = xpool.tile(
                        [128, 512], F32R, tag="x", name=f"x{which}{ec}_{sb}"
                    )
                    nc.sync.dma_start(
                        out=t[:, :],
                        in_=xt[
                            ec * 128 : (ec + 1) * 128, sb * 512 : (sb + 1) * 512
                        ],
                    )
                    x_t.append(t)
                for c in range(ndq):
                    ps = pp.tile([128, 512], F32, tag="pp", name="ps_pj")
                    for ec in range(nec):
                        nc.tensor.matmul(
                            ps[:, :],
                            r(w_t[ec][:, c * 128 : (c + 1) * 128]),
                            r(x_t[ec][:, :]),
                            start=(ec == 0),
                            stop=(ec == nec - 1),
                        )
                    nc.vector.tensor_scalar_add(
                        dst[c][:, sb * 512 : (sb + 1) * 512],
                        ps[:, :],
                        bias[:, c : c + 1],
                    )

        # ---- phase 3: V projection (natural layout [s, dv] + ones) ----
        wv_t = []
        for ec in range(nec):
            t = wpool.tile([128, dq], F32R, tag="w", name=f"wv{ec}")
            nc.sync.dma_start(
                out=t[:, :], in_=wvd[ec * 128 : (ec + 1) * 128, :]
            )
            wv_t.append(t)
        for sb in range(nxw):
            x_t = []
            for ec in range(nec):
                t = xpool.tile([128, 512], F32R, tag="x", name=f"xv{ec}_{sb}")
                nc.sync.dma_start(
                    out=t[:, :],
                    in_=xv[
                        ec * 128 : (ec + 1) * 128, sb * 512 : (sb + 1) * 512
                    ],
                )
                x_t.append(t)
            for ii in range(4):
                i = sb * 4 + ii
                ps = pp.tile([128, dq], F32, tag="pp", name="ps_v")
                for ec in range(nec):
                    nc.tensor.matmul(
                        ps[:, :],
                        r(x_t[ec][:, ii * 128 : (ii + 1) * 128]),
                        r(wv_t[ec][:, :]),
                        start=(ec == 0),
                        stop=(ec == nec - 1),
                    )
                v3 = v_sb[i].rearrange("p (h d1) -> p h d1", d1=d + 1)
                nc.vector.tensor_add(
                    v3[:, :, 0:d],
                    ps[:, :].rearrange("p (h d0) -> p h d0", d0=d),
                    bv_sb[:, :].rearrange("p (h d0) -> p h d0", d0=d),
                )
                nc.vector.tensor_copy(
                    v3[:, :, d : d + 1], onv_sb[:, :].unsqueeze(2)
                )

        # ---- phase 4: attention per (q block, head) ----
        for qb in range(nsb):
            nkb = min((qb + 1) * ndiag, nsk) if causal else nsk
            kbs = list(range(nkb))
            grps = [kbs[i : i + g] for i in range(0, len(kbs), g)]
            for h in range(hpc):
                c, hp = h // 2, (h % 2) * 64
                for wi in range(nqw):
                    q0 = qb * sq + wi * qw
                    ops_ = vp.tile([d + 1, qw], F32, tag="vo", name="ops")
                    first = True
                    for grp in grps:
                        scp = sp.tile([128, g * qw], F32, tag="sc", name="scp")
                        for i, kb in enumerate(grp):
                            nc.tensor.matmul(
                                scp[:, i * qw : (i + 1) * qw],
                                r(k_sb[c][hp : hp + d, kb * SK : (kb + 1) * SK]),
                                r(q_sb[c][hp : hp + d, q0 : q0 + qw]),
                                start=True,
                                stop=True,
                            )
                            if causal and kb >= nkb - ndiag:
                                j = kb - (nkb - ndiag)
                                m0 = (ndiag - 1 - j) * SK + wi * qw
                                nc.vector.tensor_add(
                                    scp[:, i * qw : (i + 1) * qw],
                                    scp[:, i * qw : (i + 1) * qw],
                                    mk_sb[:, m0 : m0 + qw],
                                )
                            elif not causal:
                                fm = ppool.tile(
                                    [128, qw], F32, tag="fm", name="fm"
                                )
                                nc.sync.dma_start(
                                    out=fm[:, :],
                                    in_=mkd[
                                        kb * SK : (kb + 1) * SK, q0 : q0 + qw
                                    ],
                                )
                                nc.vector.tensor_add(
                                    scp[:, i * qw : (i + 1) * qw],
                                    scp[:, i * qw : (i + 1) * qw],
                                    fm[:, :],
                                )
                        pt = ppool.tile(
                            [128, len(grp) * qw], F32R, tag="p", name="pt"
                        )
                        nc.scalar.activation(
                            pt[:, :],
                            scp[:, 0 : len(grp) * qw],
                            AF.Exp,
                            scale=float(1.0 / np.sqrt(d)),
                        )
                        for i, kb in enumerate(grp):
                            nc.tensor.matmul(
                                ops_[:, :],
                                r(
                                    v_sb[kb].rearrange(
                                        "p (h d1) -> p h d1", d1=d + 1
                                    )[:, h, :]
                                ),
                                r(pt[:, i * qw : (i + 1) * qw]),
                                start=first,
                                stop=(kb == kbs[-1]),
                            )
                            first = False
                    # normalize: psum row d holds the softmax denominator.
                    # Broadcast it across 64 partitions with a K=1 matmul
                    # against a ones vector, then reciprocal + multiply.
                    srow = nrm.tile([1, qw], F32R, tag="srow", name="srow")
                    nc.vector.tensor_copy(srow[:, :], ops_[d : d + 1, :])
                    bc = sp.tile([128, g * qw], F32, tag="sc", name="bc")
                    nc.tensor.matmul(
                        bc[0:d, 0:qw],
                        r(ones_sb[0:1, :]),
                        r(srow[0:1, :]),
                        start=True,
                        stop=True,
                    )
                    # 1/x = exp(-ln(x)) on ACT; Ln+Exp share one table set
                    # with the softmax exp, so no table reloads.
                    lnt = nrm.tile([d, qw], F32, tag="lnt", name="lnt")
                    nc.scalar.activation(lnt[:, :], bc[0:d, 0:qw], AF.Ln)
                    rb = nrm.tile([d, qw], F32, tag="rb", name="rb")
                    nc.scalar.activation(rb[:, :], lnt[:, :], AF.Exp, scale=-1.0)
                    nc.vector.tensor_mul(
                        at_sb[c][hp : hp + d, q0 : q0 + qw],
                        ops_[0:d, :],
                        rb[:, :],
                    )

        if debug:
            ndq_ = dq // 128
            for c in range(ndq_):
                nc.sync.dma_start(
                    out=dbg_q[c * 128 : (c + 1) * 128, :], in_=q_sb[c][:, :]
                )
                nc.sync.dma_start(
                    out=dbg_k[c * 128 : (c + 1) * 128, :], in_=k_sb[c][:, :]
                )
                nc.sync.dma_start(
                    out=dbg_at[c * 128 : (c + 1) * 128, :], in_=at_sb[c][:, :]
                )
            for i in range(nsc):
                nc.sync.dma_start(
                    out=dbg_v[i * 128 : (i + 1) * 128, :], in_=v_sb[i][:, :]
                )

        # ---- phase 5: output projection ----
        ow = min(e, 512)
        for ob in range(e // ow):
            wo_t = []
            for c in range(ndq):
                t = wop.tile([128, ow], F32R, tag="wo", name=f"wo{ob}_{c}")
                nc.sync.dma_start(
                    out=t[:, :],
                    in_=wod[c * 128 : (c + 1) * 128, ob * ow : (ob + 1) * ow],
                )
                wo_t.append(t)
            for i in range(nsc):
                ps = pp.tile([128, ow], F32, tag="pp", name="ps_o")
                for c in range(ndq):
                    nc.tensor.matmul(
                        ps[:, :],
                        r(at_sb[c][:, i * 128 : (i + 1) * 128]),
                        r(wo_t[c][:, :]),
                        start=(c == 0),
                        stop=(c == ndq - 1),
                    )
                ot = opool.tile([128, ow], F32, tag="o", name="ot")
                nc.vector.tensor_copy(ot[:, :], ps[:, :])
                nc.sync.dma_start(
                    out=out[i * 128 : (i + 1) * 128, ob * ow : (ob + 1) * ow],
                    in_=ot[:, :],
                )

    if split_waits:
        split_excess_waits(nc)
    return nc


def make_maskadd(sq=SQ):
    """Shifted additive mask strip [128, sq + (ndiag-1)*SK].

    Strip[kk, u] = 0 if kk <= u - (ndiag-1)*SK else NEG; pattern j is the
    window starting at col (ndiag-1-j)*SK.
    """
    ndiag = sq // SK
    w = sq + (ndiag - 1) * SK
    kk = np.arange(SK)[:, None]
    u = np.arange(w)[None, :]
    return np.where(kk <= u - (ndiag - 1) * SK, 0.0, NEG).astype(np.float32)


def prep_core_inputs(query, key, value, Wq, bq, Wk, bk, Wv, bv, Wo, bo, mask):
    """Shard + lay out host-side numpy inputs for the 8 cores."""
    causal = bool(
        np.array_equal(
            np.asarray(mask).reshape(S, S), np.tril(np.ones((S, S), bool))
        )
    )
    ma = make_maskadd() if causal else None
    mfull = (
        None
        if causal
        else np.ascontiguousarray(
            np.where(np.asarray(mask).reshape(S, S).T, 0.0, NEG).astype(
                np.float32
            )
        )
    )
    maps = []
    for core in range(NCORES):
        b, gi = core // NGROUPS, core % NGROUPS
        gs = slice(gi * DQ, (gi + 1) * DQ)
        im = {
            "xq_t": np.ascontiguousarray(np.asarray(query[b]).T),
            "xk_t": np.ascontiguousarray(np.asarray(key[b]).T),
            "xv_t": np.ascontiguousarray(np.asarray(value[b]).T),
            "wq_t": np.ascontiguousarray(np.asarray(Wq)[gs, :].T),
            "wk_t": np.ascontiguousarray(np.asarray(Wk)[gs, :].T),
            "wv_t": np.ascontiguousarray(np.asarray(Wv)[gs, :].T),
            "wo_t": np.ascontiguousarray(np.asarray(Wo)[:, gs].T),
            "bq": np.ascontiguousarray(np.asarray(bq)[gs].reshape(-1, 128).T),
            "bk": np.ascontiguousarray(np.asarray(bk)[gs].reshape(-1, 128).T),
            "bv_b": np.ascontiguousarray(
                np.broadcast_to(np.asarray(bv)[gs], (128, DQ))
            ),
            "ones_c": np.ones((1, D), dtype=np.float32),
            "ones_v": np.ones((128, HPC), dtype=np.float32),
        }
        if causal:
            im["maskadd"] = ma
        else:
            im["maskadd_full"] = mfull
        maps.append(im)
    return maps, causal


def make_runner(nc, n_cores=NCORES):
    """Build a reusable jitted SPMD executor for `nc` on cores 0..n_cores-1.

    Same execution path run_bass_kernel_spmd takes under axon
    (bass2jax -> PJRT custom call), but the jitted function is kept so
    repeated calls don't re-trigger the NEFF compile.
    """
    import jax
    from jax.experimental.shard_map import shard_map
    from jax.sharding import Mesh, PartitionSpec

    from concourse import bass2jax, mybir as _mybir

    bass2jax.install_neuronx_cc_hook()

    partition_name = (
        nc.partition_id_tensor.name if nc.partition_id_tensor else None
    )
    in_names, out_names, out_avals, zero_shapes = [], [], [], []
    for alloc in nc.m.functions[0].allocations:
        if not isinstance(alloc, _mybir.MemoryLocationSet):
            continue
        name = alloc.memorylocations[0].name
        if alloc.kind == "ExternalInput":
            if name != partition_name:
                in_names.append(name)
        elif alloc.kind == "ExternalOutput":
            out_names.append(name)
            shape = tuple(alloc.tensor_shape)
            dtype = _mybir.dt.np(alloc.dtype)
            out_avals.append(jax.core.ShapedArray(shape, dtype))
            zero_shapes.append((shape, dtype))
    n_params = len(in_names)
    all_in = list(in_names) + list(out_names)
    if partition_name is not None:
        all_in.append(partition_name)

    def _body(*args):
        operands = list(args)
        if partition_name is not None:
            operands.append(bass2jax.partition_id_tensor())
        outs = bass2jax._bass_exec_p.bind(
            *operands,
            out_avals=tuple(out_avals),
            in_names=tuple(all_in),
            out_names=tuple(out_names),
            lowering_input_output_aliases=(),
            sim_require_finite=True,
            sim_require_nnan=True,
            nc=nc,
        )
        return tuple(outs)

    devices = jax.devices()[:n_cores]
    assert len(devices) == n_cores
    mesh = Mesh(np.asarray(devices), ("core",))
    in_specs = (PartitionSpec("core"),) * (n_params + len(out_names))
    out_specs = (PartitionSpec("core"),) * len(out_names)
    sharded = jax.jit(
        shard_map(
            _body,
            mesh=mesh,
            in_specs=in_specs,
            out_specs=out_specs,
            check_rep=False,
        ),
        keep_unused=True,
    )
    zeros = [
        np.zeros((n_cores * sh[0], *sh[1:]), dt) for sh, dt in zero_shapes
    ]

    def concat_inputs(in_maps):
        return [
            np.concatenate(
                [np.asarray(in_maps[c][n]) for c in range(n_cores)], axis=0
            )
            for n in in_names
        ]

    def run(in_maps):
        out_arrs = sharded(*concat_inputs(in_maps), *zeros)
        return [
            {
                name: np.asarray(out_arrs[i]).reshape(
                    n_cores, *out_avals[i].shape
                )[c]
                for i, name in enumerate(out_names)
            }
            for c in range(n_cores)
        ]

    run.sharded = sharded
    run.concat_inputs = concat_inputs
    run.zeros = zeros
    run.out_names = out_names
    run.out_avals = out_avals
    return run


_CACHE = {}


def get_runner(causal=True):
    ck = "causal" if causal else "dense"
    if ck not in _CACHE:
        nc = build_kernel(causal=causal)
        _CACHE[ck] = make_runner(nc)
    return _CACHE[ck]


def kernel(**inputs) -> np.ndarray:
    in_maps, causal = prep_core_inputs(**inputs)
    run = get_runner(causal)
    results = run(in_maps)
    bo = np.asarray(inputs["bo"], dtype=np.float32)
    out = np.empty((B, S, E), dtype=np.float32)
    for b in range(B):
        acc = results[b * NGROUPS]["out"].astype(np.float32).copy()
        for gi in range(1, NGROUPS):
            acc += results[b * NGROUPS + gi]["out"]
        out[b] = acc + bo[None, :]
    return out


# revision 35
# speedup vs baseline: 71.3274x; 71.3274x over previous
"""Trainium2 Bass kernel: 16-head causal attention (B=4, S=2048, E=1024).

Sharding: 8 cores = 4 batches x 2 head-groups (8 heads each).
Per-core, fp32 storage with float32r (FP22) matmuls at full PE rate:
  - q^T = Wq_g X^T, k^T = Wk_g X^T   (transposed projections, [dq, S])
  - V = X^T.T Wv_g^T                 (natural layout [S, dv], + ones column
    per head so the PV matmul also produces softmax denominators)
  - scores^T[k, q] blocks = k^T.T q^T (K=64 contraction, fp32r, N=512)
  - P^T = exp(scores^T/8 + causal additive mask)  (no max subtraction:
    scores are O(10), fp32 exp is safe; masked lanes underflow to 0)
  - out^T[d+1, q] += V_aug^T P^T accumulated in PSUM across k blocks;
    row d is the denominator. Normalize with a broadcast reciprocal.
  - partial = attnT.T @ Wo_g^T; host sums the two head-group partials + bo.
Causal: fully-masked k blocks are skipped; only the diagonal band gets
additive mask tiles (4 host-precomputed patterns).
"""

import contextlib

import numpy as np

import bass_rust
import concourse.bass as bass
import concourse.mybir as mybir
import concourse.tile as tile
from concourse.bass_utils import run_bass_kernel_spmd

F32 = mybir.dt.float32
F32R = mybir.dt.float32r
AF = mybir.ActivationFunctionType

B, S, E = 4, 2048, 1024
H, D = 16, 64
NCORES = 8
NGROUPS = 2            # head groups (tensor parallel)
HPC = H // NGROUPS     # heads per core
DQ = HPC * D           # per-core projection width = 512
NEG = -8.0e9           # additive mask; *0.125 in exp -> -1e9 -> exp == 0.0

SQ = 512               # q tile (free dim of scores^T)
SK = 128               # k block (partition dim of scores^T)
G = 2                  # k blocks per exp group (psum [128, G*512])


def r(ap):
    """View an fp32 AP as float32r for full-rate PE matmuls."""
    return ap.bitcast(F32R)


def split_excess_waits(nc, maxw=1):
    """This container's walrus supports one sem wait per instruction;
    hoist extras onto same-engine nops just before the instruction."""
    n_new = 0
    for bb in nc.main_func.blocks:
        new_list = []
        changed = False
        for inst in list(bb.instructions):
            si = inst.sync_info
            waits = list(si.on_wait) if si and si.on_wait else []
            if len(waits) > maxw:
                changed = True
                extra, keep = waits[:-maxw], waits[-maxw:]
                for ci in range(0, len(extra), maxw):
                    nop = bass_rust.InstNoOp(
                        name=f"I-waitsplit-{n_new}", ins=[], outs=[]
                    )
                    n_new += 1
                    nop.engine = inst.engine
                    nop.sync_info = mybir.SyncInfo(
                        on_wait=extra[ci : ci + maxw], on_update=[]
                    )
                    new_list.append(nop)
                inst.sync_info = mybir.SyncInfo(
                    on_wait=keep,
                    on_update=list(si.on_update) if si.on_update else [],
                )
            new_list.append(inst)
        if changed:
            bb.instructions = new_list
    return n_new


def build_kernel(s=S, e=E, hpc=HPC, d=D, causal=True, sq=SQ, g=G,
                 split_waits=True, debug=False, reps=1):
    dq = hpc * d
    nec = e // 128            # input-feature chunks
    ndq = dq // 128           # projection partition chunks
    nsb = s // sq             # q blocks
    nsk = s // SK             # k blocks
    nsc = s // 128            # s chunks of 128
    ndiag = sq // SK          # diagonal mask patterns
    qw = min(sq, 512)         # matmul moving width
    nqw = sq // qw
    nxw = s // 512            # x piece columns

    nc = bass.Bass()

    xq = nc.declare_dram_parameter("xq_t", [e, s], F32R, isOutput=False)
    xk = nc.declare_dram_parameter("xk_t", [e, s], F32R, isOutput=False)
    xv = nc.declare_dram_parameter("xv_t", [e, s], F32R, isOutput=False)
    wqd = nc.declare_dram_parameter("wq_t", [e, dq], F32R, isOutput=False)
    wkd = nc.declare_dram_parameter("wk_t", [e, dq], F32R, isOutput=False)
    wvd = nc.declare_dram_parameter("wv_t", [e, dq], F32R, isOutput=False)
    wod = nc.declare_dram_parameter("wo_t", [dq, e], F32R, isOutput=False)
    bqd = nc.declare_dram_parameter("bq", [128, ndq], F32, isOutput=False)
    bkd = nc.declare_dram_parameter("bk", [128, ndq], F32, isOutput=False)
    bvd = nc.declare_dram_parameter("bv_b", [128, dq], F32, isOutput=False)
    if causal:
        mkd = nc.declare_dram_parameter(
            "maskadd", [128, sq + (ndiag - 1) * SK], F32R, isOutput=False
        )
    else:
        mkd = nc.declare_dram_parameter(
            "maskadd_full", [s, s], F32, isOutput=False
        )
    onc = nc.declare_dram_parameter("ones_c", [1, d], F32R, isOutput=False)
    idd = nc.declare_dram_parameter("ident", [128, 128], F32R, isOutput=False)
    onv = nc.declare_dram_parameter("ones_v", [128, hpc], F32R, isOutput=False)
    out = nc.declare_dram_parameter("out", [s, e], F32, isOutput=True)
    if debug:
        dbg_q = nc.declare_dram_parameter("dbg_q", [dq, s], F32, isOutput=True)
        dbg_k = nc.declare_dram_parameter("dbg_k", [dq, s], F32, isOutput=True)
        dbg_v = nc.declare_dram_parameter(
            "dbg_v", [s, hpc * (d + 1)], F32, isOutput=True
        )
        dbg_at = nc.declare_dram_parameter(
            "dbg_at", [dq, s], F32, isOutput=True
        )

    with tile.TileContext(nc) as tc, contextlib.ExitStack() as ctx:
        pers = ctx.enter_context(tc.tile_pool(name="pers", bufs=1))
        wpool = ctx.enter_context(tc.tile_pool(name="wp", bufs=nec + 1))
        xpool = ctx.enter_context(tc.tile_pool(name="xp", bufs=nec + 1))
        wop = ctx.enter_context(tc.tile_pool(name="wop", bufs=2 * ndq))
        ppool = ctx.enter_context(tc.tile_pool(name="ppl", bufs=2))
        nrm = ctx.enter_context(tc.tile_pool(name="nrm", bufs=2))
        opool = ctx.enter_context(tc.tile_pool(name="opl", bufs=2))
        pp = ctx.enter_context(tc.tile_pool(name="pp", bufs=2, space="PSUM"))
        sp = ctx.enter_context(tc.tile_pool(name="sp", bufs=2, space="PSUM"))
        vp = ctx.enter_context(tc.tile_pool(name="vp", bufs=2, space="PSUM"))

        # ---- constants / persistent tensors ----
        bq_sb = pers.tile([128, ndq], F32, name="bq_sb")
        nc.sync.dma_start(out=bq_sb[:, :], in_=bqd[:, :])
        bk_sb = pers.tile([128, ndq], F32, name="bk_sb")
        nc.sync.dma_start(out=bk_sb[:, :], in_=bkd[:, :])
        bv_sb = pers.tile([128, dq], F32, name="bv_sb")
        nc.sync.dma_start(out=bv_sb[:, :], in_=bvd[:, :])
        ones_sb = pers.tile([1, d], F32R, name="ones_sb")
        nc.sync.dma_start(out=ones_sb[:, :], in_=onc[:, :])
        onv_sb = pers.tile([128, hpc], F32R, name="onv_sb")
        nc.sync.dma_start(out=onv_sb[:, :], in_=onv[:, :])
        id_sb = pers.tile([128, 128], F32R, name="id_sb")
        nc.sync.dma_start(out=id_sb[:, :], in_=idd[:, :])
        if causal:
            # one shifted strip: pattern j at cols (ndiag-1-j)*SK
            mk_sb = pers.tile([128, sq + (ndiag - 1) * SK], F32R, name="mk_sb")
            nc.sync.dma_start(out=mk_sb[:, :], in_=mkd[:, :])

        q_sb = [
            [pers.tile([128, 512], F32R, name=f"q_sb{c}_{w}")
             for w in range(nxw)]
            for c in range(ndq)
        ]
        k_sb = [
            [pers.tile([128, 512], F32R, name=f"k_sb{c}_{w}")
             for w in range(nxw)]
            for c in range(ndq)
        ]
        v_sb = [
            pers.tile([128, hpc * (d + 1)], F32R, name=f"v_sb{i}")
            for i in range(nsc)
        ]
        at_sb = [
            [pers.tile([128, 512], F32R, name=f"at_sb{c}_{w}")
             for w in range(nxw)]
            for c in range(ndq)
        ]

        # ---- helpers ----
        def load_w(wt, tagname):
            ts_ = []
            for ec in range(nec):
                t = wpool.tile([128, dq], F32R, tag="w", name=f"{tagname}{ec}")
                nc.sync.dma_start(
                    out=t[:, :], in_=wt[ec * 128 : (ec + 1) * 128, :]
                )
                ts_.append(t)
            return ts_

        def load_x(xt, sb, tagname):
            ts_ = []
            for ec in range(nec):
                t = xpool.tile([128, 512], F32R, tag="x",
                               name=f"{tagname}{ec}_{sb}")
                nc.sync.dma_start(
                    out=t[:, :],
                    in_=xt[ec * 128 : (ec + 1) * 128,
                           sb * 512 : (sb + 1) * 512],
                )
                ts_.append(t)
            return ts_

        def proj_qk_col(w_t, x_t, dst, bias, sb):
            """One 512-col slab of a transposed projection."""
            for c in range(ndq):
                ps = pp.tile([128, 512], F32, tag="pp", name="ps_pj")
                for ec in range(nec):
                    nc.tensor.matmul(
                        ps[:, :],
                        r(w_t[ec][:, c * 128 : (c + 1) * 128]),
                        r(x_t[ec][:, :]),
                        start=(ec == 0),
                        stop=(ec == nec - 1),
                    )
                nc.vector.tensor_scalar_add(
                    dst[c][sb][:, :], ps[:, :], bias[:, c : c + 1]
                )

        def proj_v_col(wv_t, x_t, sb):
            for ii in range(4):
                i = sb * 4 + ii
                ps = pp.tile([128, dq], F32, tag="pp", name="ps_v")
                for ec in range(nec):
                    nc.tensor.matmul(
                        ps[:, :],
                        r(x_t[ec][:, ii * 128 : (ii + 1) * 128]),
                        r(wv_t[ec][:, :]),
                        start=(ec == 0),
                        stop=(ec == nec - 1),
                    )
                v3 = v_sb[i].rearrange("p (h d1) -> p h d1", d1=d + 1)
                nc.vector.tensor_add(
                    v3[:, :, 0:d],
                    ps[:, :].rearrange("p (h d0) -> p h d0", d0=d),
                    bv_sb[:, :].rearrange("p (h d0) -> p h d0", d0=d),
                )
                nc.vector.tensor_copy(
                    v3[:, :, d : d + 1], onv_sb[:, :].unsqueeze(2)
                )

        def attention_qb(qb):
            nkb = min((qb + 1) * ndiag, nsk) if causal else nsk
            kbs = list(range(nkb))
            grps = [kbs[i : i + g] for i in range(0, len(kbs), g)]
            for h in range(hpc):
                c, hp = h // 2, (h % 2) * 64
                for wi in range(nqw):
                    q0 = qb * sq + wi * qw
                    ops_ = vp.tile([d + 1, qw], F32, tag="vo", name="ops")
                    first = True
                    for grp in grps:
                        scp = sp.tile([128, g * qw], F32, tag="sc", name="scp")
                        for i, kb in enumerate(grp):
                            diag = causal and kb >= nkb - ndiag
                            nc.tensor.matmul(
                                scp[:, i * qw : (i + 1) * qw],
                                r(
                                    k_sb[c][kb // 4][
                                        hp : hp + d,
                                        (kb % 4) * SK : (kb % 4 + 1) * SK,
                                    ]
                                ),
                                r(
                                    q_sb[c][q0 // 512][
                                        hp : hp + d, q0 % 512 : q0 % 512 + qw
                                    ]
                                ),
                                start=True,
                                stop=not diag,
                            )
                            if diag:
                                # accumulate additive causal mask via PE
                                j = kb - (nkb - ndiag)
                                m0 = (ndiag - 1 - j) * SK + wi * qw
                                nc.tensor.matmul(
                                    scp[:, i * qw : (i + 1) * qw],
                                    r(id_sb[:, :]),
                                    mk_sb[:, m0 : m0 + qw],
                                    start=False,
                                    stop=True,
                                )
                            elif not causal:
                                fm = ppool.tile(
                                    [128, qw], F32, tag="fm", name="fm"
                                )
                                nc.sync.dma_start(
                                    out=fm[:, :],
                                    in_=mkd[
                                        kb * SK : (kb + 1) * SK, q0 : q0 + qw
                                    ],
                                )
                                nc.vector.tensor_add(
                                    scp[:, i * qw : (i + 1) * qw],
                                    scp[:, i * qw : (i + 1) * qw],
                                    fm[:, :],
                                )
                        pt = ppool.tile(
                            [128, len(grp) * qw], F32R, tag="p", name="pt"
                        )
                        nc.scalar.activation(
                            pt[:, :],
                            scp[:, 0 : len(grp) * qw],
                            AF.Exp,
                            scale=float(1.0 / np.sqrt(d)),
                        )
                        for i, kb in enumerate(grp):
                            nc.tensor.matmul(
                                ops_[:, :],
                                r(
                                    v_sb[kb].rearrange(
                                        "p (h d1) -> p h d1", d1=d + 1
                                    )[:, h, :]
                                ),
                                r(pt[:, i * qw : (i + 1) * qw]),
                                start=first,
                                stop=(kb == kbs[-1]),
                            )
                            first = False
                    # Copy numerators + denominator row out immediately so
                    # the PV accumulator bank frees before the normalize
                    # chain (DVE->PE->ACT->DVE) completes.
                    atw = at_sb[c][q0 // 512]
                    aw0 = q0 % 512
                    srow = nrm.tile([1, qw], F32R, tag="srow", name="srow")
                    nc.vector.tensor_copy(srow[:, :], ops_[d : d + 1, :])
                    # broadcast denominators across 64 partitions (K=1 mm)
                    bc = pp.tile([128, qw], F32, tag="pp", name="bc")
                    nc.tensor.matmul(
                        bc[0:d, 0:qw],
                        r(ones_sb[0:1, :]),
                        r(srow[0:1, :]),
                        start=True,
                        stop=True,
                    )
                    # 1/x = exp(-ln(x)) on ACT; Ln+Exp share one table set
                    # with the softmax exp, so no table reloads.
                    nc.scalar.activation(bc[0:d, 0:qw], bc[0:d, 0:qw], AF.Ln)
                    rb = nrm.tile([d, qw], F32, tag="rb", name="rb")
                    nc.scalar.activation(rb[:, :], bc[0:d, 0:qw], AF.Exp,
                                         scale=-1.0)
                    nc.vector.tensor_mul(
                        atw[hp : hp + d, aw0 : aw0 + qw],
                        ops_[0:d, :],
                        rb[:, :],
                    )

        ow = min(e, 512)

        spc = sq // 128  # s-chunks per q block

        def wo_qb(qb, wo_t):
            for i in range(qb * spc, (qb + 1) * spc):
                for ob in range(e // ow):
                    ps = pp.tile([128, ow], F32, tag="pp", name="ps_o")
                    for c in range(ndq):
                        nc.tensor.matmul(
                            ps[:, :],
                            r(
                                at_sb[c][i // 4][
                                    :, (i % 4) * 128 : (i % 4 + 1) * 128
                                ]
                            ),
                            r(wo_t[ob][c][:, :]),
                            start=(c == 0),
                            stop=(c == ndq - 1),
                        )
                    ot = opool.tile([128, ow], F32, tag="o", name="ot")
                    nc.vector.tensor_copy(ot[:, :], ps[:, :])
                    nc.sync.dma_start(
                        out=out[
                            i * 128 : (i + 1) * 128, ob * ow : (ob + 1) * ow
                        ],
                        in_=ot[:, :],
                    )

        def emit_body(rep):
            wq_t = load_w(wqd, f"r{rep}wq")
            for sb in range(nxw):
                x_t = load_x(xq, sb, f"r{rep}xq")
                proj_qk_col(wq_t, x_t, q_sb, bq_sb, sb)

            wo_t = []
            for ob in range(e // ow):
                row = []
                for c in range(ndq):
                    t = wop.tile([128, ow], F32R, tag="wo",
                                 name=f"r{rep}wo{ob}_{c}")
                    nc.sync.dma_start(
                        out=t[:, :],
                        in_=wod[
                            c * 128 : (c + 1) * 128, ob * ow : (ob + 1) * ow
                        ],
                    )
                    row.append(t)
                wo_t.append(row)

            slab_done = 0
            for qb in range(nsb):
                # prefetch one q-block ahead so slab projections overlap
                # the previous block's ACT-bound attention
                need = (
                    min(-(-((qb + 2) * sq) // 512), nxw) if causal else nxw
                )
                while slab_done < need:
                    sb = slab_done
                    wv_t = load_w(wvd, f"r{rep}wv{sb}_")
                    xv_t = load_x(xv, sb, f"r{rep}xv{sb}_")
                    proj_v_col(wv_t, xv_t, sb)
                    wk_t = load_w(wkd, f"r{rep}wk{sb}_")
                    xk_t = load_x(xk, sb, f"r{rep}xk{sb}_")
                    proj_qk_col(wk_t, xk_t, k_sb, bk_sb, sb)
                    slab_done += 1
                attention_qb(qb)
                wo_qb(qb, wo_t)

        # ---- emission: q proj, then per-qb (V slab, K slab, attention,
        # Wo) so ACT work becomes available early and Wo overlaps the
        # next q-block's attention ----
        for _rep in range(reps):
            emit_body(_rep)

        if debug:
            for c in range(ndq):
                for w in range(nxw):
                    cs = slice(c * 128, (c + 1) * 128)
                    ws = slice(w * 512, (w + 1) * 512)
                    nc.sync.dma_start(out=dbg_q[cs, ws], in_=q_sb[c][w][:, :])
                    nc.sync.dma_start(out=dbg_k[cs, ws], in_=k_sb[c][w][:, :])
                    nc.sync.dma_start(
                        out=dbg_at[cs, ws], in_=at_sb[c][w][:, :]
                    )
            for i in range(nsc):
                nc.sync.dma_start(
                    out=dbg_v[i * 128 : (i + 1) * 128, :], in_=v_sb[i][:, :]
                )

    if split_waits:
        split_excess_waits(nc)
    return nc


def make_maskadd(sq=SQ):
    """Shifted additive mask strip [128, sq + (ndiag-1)*SK].

    Strip[kk, u] = 0 if kk <= u - (ndiag-1)*SK else NEG; pattern j is the
    window starting at col (ndiag-1-j)*SK.
    """
    ndiag = sq // SK
    w = sq + (ndiag - 1) * SK
    kk = np.arange(SK)[:, None]
    u = np.arange(w)[None, :]
    return np.where(kk <= u - (ndiag - 1) * SK, 0.0, NEG).astype(np.float32)


def prep_core_inputs(query, key, value, Wq, bq, Wk, bk, Wv, bv, Wo, bo, mask):
    """Shard + lay out host-side numpy inputs for the 8 cores."""
    causal = bool(
        np.array_equal(
            np.asarray(mask).reshape(S, S), np.tril(np.ones((S, S), bool))
        )
    )
    ma = make_maskadd() if causal else None
    mfull = (
        None
        if causal
        else np.ascontiguousarray(
            np.where(np.asarray(mask).reshape(S, S).T, 0.0, NEG).astype(
                np.float32
            )
        )
    )
    maps = []
    for core in range(NCORES):
        b, gi = core // NGROUPS, core % NGROUPS
        gs = slice(gi * DQ, (gi + 1) * DQ)
        im = {
            "xq_t": np.ascontiguousarray(np.asarray(query[b]).T),
            "xk_t": np.ascontiguousarray(np.asarray(key[b]).T),
            "xv_t": np.ascontiguousarray(np.asarray(value[b]).T),
            "wq_t": np.ascontiguousarray(np.asarray(Wq)[gs, :].T),
            "wk_t": np.ascontiguousarray(np.asarray(Wk)[gs, :].T),
            "wv_t": np.ascontiguousarray(np.asarray(Wv)[gs, :].T),
            "wo_t": np.ascontiguousarray(np.asarray(Wo)[:, gs].T),
            "bq": np.ascontiguousarray(np.asarray(bq)[gs].reshape(-1, 128).T),
            "bk": np.ascontiguousarray(np.asarray(bk)[gs].reshape(-1, 128).T),
            "bv_b": np.ascontiguousarray(
                np.broadcast_to(np.asarray(bv)[gs], (128, DQ))
            ),
            "ones_c": np.ones((1, D), dtype=np.float32),
            "ones_v": np.ones((128, HPC), dtype=np.float32),
            "ident": np.eye(128, dtype=np.float32),
        }
        if causal:
            im["maskadd"] = ma
        else:
            im["maskadd_full"] = mfull
        maps.append(im)
    return maps, causal


def make_runner(nc, n_cores=NCORES):
    """Build a reusable jitted SPMD executor for `nc` on cores 0..n_cores-1.

    Same execution path run_bass_kernel_spmd takes under axon
    (bass2jax -> PJRT custom call), but the jitted function is kept so
    repeated calls don't re-trigger the NEFF compile.
    """
    import jax
    from jax.experimental.shard_map import shard_map
    from jax.sharding import Mesh, PartitionSpec

    from concourse import bass2jax, mybir as _mybir

    bass2jax.install_neuronx_cc_hook()

    partition_name = (
        nc.partition_id_tensor.name if nc.partition_id_tensor else None
    )
    in_names, out_names, out_avals, zero_shapes = [], [], [], []
    for alloc in nc.m.functions[0].allocations:
        if not isinstance(alloc, _mybir.MemoryLocationSet):
            continue
        name = alloc.memorylocations[0].name
        if alloc.kind == "ExternalInput":
            if name != partition_name:
                in_names.append(name)
        elif alloc.kind == "ExternalOutput":
            out_names.append(name)
            shape = tuple(alloc.tensor_shape)
            dtype = _mybir.dt.np(alloc.dtype)
            out_avals.append(jax.core.ShapedArray(shape, dtype))
            zero_shapes.append((shape, dtype))
    n_params = len(in_names)
    all_in = list(in_names) + list(out_names)
    if partition_name is not None:
        all_in.append(partition_name)

    def _body(*args):
        operands = list(args)
        if partition_name is not None:
            operands.append(bass2jax.partition_id_tensor())
        outs = bass2jax._bass_exec_p.bind(
            *operands,
            out_avals=tuple(out_avals),
            in_names=tuple(all_in),
            out_names=tuple(out_names),
            lowering_input_output_aliases=(),
            sim_require_finite=True,
            sim_require_nnan=True,
            nc=nc,
        )
        return tuple(outs)

    devices = jax.devices()[:n_cores]
    assert len(devices) == n_cores
    mesh = Mesh(np.asarray(devices), ("core",))
    in_specs = (PartitionSpec("core"),) * (n_params + len(out_names))
    out_specs = (PartitionSpec("core"),) * len(out_names)
    sharded = jax.jit(
        shard_map(
            _body,
            mesh=mesh,
            in_specs=in_specs,
            out_specs=out_specs,
            check_rep=False,
        ),
        keep_unused=True,
    )
    zeros = [
        np.zeros((n_cores * sh[0], *sh[1:]), dt) for sh, dt in zero_shapes
    ]

    def concat_inputs(in_maps):
        return [
            np.concatenate(
                [np.asarray(in_maps[c][n]) for c in range(n_cores)], axis=0
            )
            for n in in_names
        ]

    def run(in_maps):
        out_arrs = sharded(*concat_inputs(in_maps), *zeros)
        return [
            {
                name: np.asarray(out_arrs[i]).reshape(
                    n_cores, *out_avals[i].shape
                )[c]
                for i, name in enumerate(out_names)
            }
            for c in range(n_cores)
        ]

    run.sharded = sharded
    run.concat_inputs = concat_inputs
    run.zeros = zeros
    run.out_names = out_names
    run.out_avals = out_avals
    return run


_CACHE = {}


def get_runner(causal=True):
    ck = "causal" if causal else "dense"
    if ck not in _CACHE:
        nc = build_kernel(causal=causal)
        _CACHE[ck] = make_runner(nc)
    return _CACHE[ck]


def kernel(**inputs) -> np.ndarray:
    in_maps, causal = prep_core_inputs(**inputs)
    run = get_runner(causal)
    results = run(in_maps)
    bo = np.asarray(inputs["bo"], dtype=np.float32)
    out = np.empty((B, S, E), dtype=np.float32)
    for b in range(B):
        acc = results[b * NGROUPS]["out"].astype(np.float32).copy()
        for gi in range(1, NGROUPS):
            acc += results[b * NGROUPS + gi]["out"]
        out[b] = acc + bo[None, :]
    return out
